# revision 6
# baseline (speedup 1.0000x reference)
"""AttnDecoderRNN single-step decoder on 8 Trainium2 NeuronCores.

Sharding strategy (tensor parallel on every output dim, vocab-TP on the
out projection):
  - attention (attn_W @ cat1, softmax-weighted sum of encoder_outputs) is
    replicated on every core (it is small and sits on the critical path;
    replication avoids an AllReduce).
  - comb / GRU weights are sharded by output rows: core k owns H-slice
    j = [128k, 128k+128). Two tiny AllGathers ([1,128] -> [1,1024]) stitch
    the hidden state back together (after comb+relu, and after the GRU).
  - out_W is sharded by vocab rows (6400 padded rows per core). Each core
    computes its logit shard plus a local sum(exp(z)); the host epilogue
    combines the 8 partial sums into log Z and subtracts (log_softmax).

Matvecs run as DVE tensor_mul (weights-tile x broadcast-vector) piped into
ScalarE activation(Copy, accum_out=) which reduces along the free dim on a
different engine, so the multiply and the reduction overlap.

Row->partition mapping is "slab" order: partition p owns contiguous DRAM
rows (50 vocab rows, 4 attention rows, 3 gate rows), so every weight DMA
is 128 large contiguous descriptors.

A dummy warmup AllGather fires at kernel start so the ncfw collective
cold-start (~25us) overlaps the weight streaming instead of sitting on the
critical path.
"""

import os
import numpy as np

import concourse.bass as bass
import concourse.bacc as bacc
import concourse.mybir as mybir
import concourse.tile as tile
from concourse.bass_utils import run_bass_kernel_spmd

F32 = mybir.dt.float32
AF = mybir.ActivationFunctionType
ALU = mybir.AluOpType

V, E, H, L = 50257, 512, 1024, 512
EH = E + H
NCORES = 8
VP = 51200            # vocab padded to 8 * 6400
VK = VP // NCORES     # 6400 vocab rows per core
TK = VK // 128        # 50 vocab rows per partition per core
DBLK = 5              # rows-per-partition fetched per big DMA
NDMA = TK // DBLK     # 10 big DMAs per core

_CACHE = {}
LAST_EXEC_NS = None


def _install_axon_profile_hook():
    """Provide antenv.axon_hooks (NTFF profiling via the axon .so) when the
    image lacks it, so run_bass_kernel_spmd(trace=True) can measure HW time.
    No-op if unavailable; the untraced path never needs it."""
    import sys
    import types
    import ctypes
    import contextlib

    if "antenv.axon_hooks" in sys.modules:
        return
    so_path = "/opt/axon/libaxon_pjrt.so"
    if not os.path.exists(so_path):
        return
    try:
        lib = ctypes.CDLL(so_path)
        if not hasattr(lib, "axon_start_nrt_profile"):
            return
        lib.axon_start_nrt_profile.argtypes = [ctypes.POINTER(ctypes.c_int64),
                                               ctypes.c_size_t]
        lib.axon_start_nrt_profile.restype = ctypes.c_int64
        lib.axon_stop_nrt_profile.argtypes = [ctypes.c_char_p]
        lib.axon_stop_nrt_profile.restype = ctypes.c_int64

        @contextlib.contextmanager
        def _hook(output_dir, device_ids):
            import jax
            jax.devices()
            if device_ids:
                ids = (ctypes.c_int64 * len(device_ids))(*device_ids)
                rc = lib.axon_start_nrt_profile(ids, len(device_ids))
            else:
                rc = lib.axon_start_nrt_profile(None, 0)
            if rc != 0:
                raise RuntimeError(f"axon_start_nrt_profile rc={rc}")
            try:
                yield
            finally:
                n = lib.axon_stop_nrt_profile(str(output_dir).encode())
                print(f"profile: {n} file(s) written to {output_dir}", file=sys.stderr)

        mod = types.ModuleType("antenv.axon_hooks")
        mod.get_axon_ntff_profile_hook = lambda: _hook
        mod.set_axon_ntff_profile_hook = lambda h: None
        sys.modules["antenv.axon_hooks"] = mod

        import concourse.bass_utils as bu
        bu.upload_artifacts = lambda tmpdir: tmpdir
    except Exception:
        pass


def _build():
    nc = bacc.Bacc("TRN2", target_bir_lowering=False, debug=False,
                   num_devices=NCORES)

    wout = nc.dram_tensor("wout", [VK, H], F32, kind="ExternalInput")
    outb = nc.dram_tensor("outb", [128, TK], F32, kind="ExternalInput")
    wih = nc.dram_tensor("wih", [384, H], F32, kind="ExternalInput")
    whh = nc.dram_tensor("whh", [384, H], F32, kind="ExternalInput")
    combw = nc.dram_tensor("combw", [128, EH], F32, kind="ExternalInput")
    srows = nc.dram_tensor("srows", [10, 128], F32, kind="ExternalInput")
    attw = nc.dram_tensor("attw", [L, EH], F32, kind="ExternalInput")
    enc = nc.dram_tensor("enc", [L, H], F32, kind="ExternalInput")
    cat1 = nc.dram_tensor("cat1", [1, EH], F32, kind="ExternalInput")
    eye = nc.dram_tensor("eye", [128, 128], F32, kind="ExternalInput")

    z_out = nc.dram_tensor("z", [128, TK], F32, kind="ExternalOutput")
    s_out = nc.dram_tensor("s", [1, 1], F32, kind="ExternalOutput")
    h1_out = nc.dram_tensor("h1f", [1, H], F32, kind="ExternalOutput")
    aw_out = nc.dram_tensor("aw", [128, 4], F32, kind="ExternalOutput")

    rg = [list(range(NCORES))]

    with tile.TileContext(nc) as tc:
        with (
            tc.tile_pool(name="wts", bufs=1) as wts,
            tc.tile_pool(name="big", bufs=3) as big,
            tc.tile_pool(name="scr", bufs=3) as scr,
            tc.tile_pool(name="cols", bufs=1) as cols,
            tc.tile_pool(name="ps", bufs=1, space="PSUM") as ps,
            tc.tile_pool(name="dram", bufs=1, space="DRAM") as dram,
        ):
            # ---- warmup collective: absorbs ncfw cold-start off the
            # critical path; result unused ----
            wu_sb = cols.tile([1, 8], F32)
            nc.gpsimd.memset(wu_sb[:], 0.0)
            wu_in = dram.tile([1, 8], F32)
            nc.gpsimd.dma_start(wu_in[:], wu_sb[:])
            wu_out = dram.tile([1, 8 * NCORES], F32, addr_space="Shared")
            nc.gpsimd.collective_compute("AllGather", ALU.bypass, replica_groups=rg,
                                         ins=[wu_in[:]], outs=[wu_out[:]])

            # ---- small weight loads (sync HWDGE ring, ahead of wout) ----
            # slab row order: partition p <- rows p*4+t (attn/enc), p*3+g (gru)
            attn_sb = wts.tile([128, 4 * EH], F32)
            attw_v = attw[:].rearrange("(p t) f -> p t f", p=128)
            for t in range(4):
                nc.sync.dma_start(attn_sb[:, t * EH:(t + 1) * EH], attw_v[:, t, :])
            enc_sb = wts.tile([128, 4 * H], F32)
            nc.sync.dma_start(enc_sb[:].rearrange("p (t h) -> p t h", t=4),
                              enc[:].rearrange("(p t) h -> p t h", p=128))
            whh_sb = wts.tile([128, 3 * H], F32)
            nc.sync.dma_start(whh_sb[:].rearrange("p (g h) -> p g h", g=3),
                              whh[:].rearrange("(p g) h -> p g h", p=128))
            wih_sb = wts.tile([128, 3 * H], F32)
            nc.sync.dma_start(wih_sb[:].rearrange("p (g h) -> p g h", g=3),
                              wih[:].rearrange("(p g) h -> p g h", p=128))
            comb_sb = wts.tile([128, EH], F32)
            nc.sync.dma_start(comb_sb[:], combw[:])

            # ---- tiny loads / consts (scalar HWDGE = low latency) ----
            cat1_bc = wts.tile([128, EH], F32)
            nc.scalar.dma_start(cat1_bc[:], cat1[:].squeeze(0).partition_broadcast(128))
            eye_sb = cols.tile([128, 128], F32)
            nc.scalar.dma_start(eye_sb[:], eye[:])
            srows_sb = cols.tile([10, 128], F32)
            nc.scalar.dma_start(srows_sb[:], srows[:])
            outb_sb = cols.tile([128, TK], F32)
            nc.scalar.dma_start(outb_sb[:], outb[:])
            ones_col = cols.tile([128, 1], F32)
            nc.gpsimd.memset(ones_col[:], 1.0)
            ones_row = cols.tile([1, 128], F32)
            nc.gpsimd.memset(ones_row[:], 1.0)

            # bias rows -> per-partition columns
            colsT_ps = ps.tile([128, 10], F32, tag="pscol", bufs=3)
            nc.tensor.transpose(colsT_ps[:], srows_sb[:], eye_sb[0:10, 0:10])
            bias_sb = cols.tile([128, 10], F32)
            nc.vector.tensor_copy(bias_sb[:], colsT_ps[:])

            # ---- attention matvec: za[p,t] = attn_W[4p+t] . cat1 ----
            zaraw = cols.tile([128, 4], F32)
            dump = cols.tile([128, 1], F32)
            for t in range(4):
                prod = scr.tile([128, EH], F32, tag="prod", name=f"prod_a{t}")
                nc.vector.tensor_mul(prod[:], attn_sb[:, t * EH:(t + 1) * EH], cat1_bc[:])
                nc.scalar.activation(dump.broadcast_to(prod[:].shape), prod[:],
                                     AF.Copy, accum_out=zaraw[:, t:t + 1])

            # ---- gh matvecs early (independent of attention/AllGather) ----
            gcols = cols.tile([128, 6], F32)   # gi r/z/n, gh r/z/n
            for g in range(3):
                prod = scr.tile([128, H], F32, tag="prod", name=f"prod_gh{g}")
                nc.vector.tensor_mul(prod[:], whh_sb[:, g * H:(g + 1) * H],
                                     cat1_bc[:, E:EH])
                nc.scalar.activation(dump.broadcast_to(prod[:].shape), prod[:],
                                     AF.Copy, accum_out=gcols[:, 3 + g:4 + g])
            # comb embedding-half early too
            x0 = cols.tile([128, 1], F32)
            prod = scr.tile([128, E], F32, tag="prod", name="prod_c0")
            nc.vector.tensor_mul(prod[:], comb_sb[:, 0:E], cat1_bc[:, 0:E])
            nc.scalar.activation(dump.broadcast_to(prod[:].shape), prod[:],
                                 AF.Copy, accum_out=x0[:])

            # ---- attention softmax-weighted sum ----
            e_tile = cols.tile([128, 4], F32)
            for t in range(4):
                nc.scalar.activation(e_tile[:, t:t + 1], zaraw[:, t:t + 1], AF.Exp,
                                     bias=bias_sb[:, 6 + t:7 + t])
            num_ps = ps.tile([1, H], F32, tag="pswide", bufs=2)
            den_ps = ps.tile([1, 1], F32, tag="pscol", bufs=3)
            for t in range(4):
                fl = (t == 0)
                ll = (t == 3)
                nc.tensor.matmul(num_ps[0:1, 0:512], e_tile[:, t:t + 1],
                                 enc_sb[:, t * H:t * H + 512], start=fl, stop=ll)
                nc.tensor.matmul(num_ps[0:1, 512:1024], e_tile[:, t:t + 1],
                                 enc_sb[:, t * H + 512:(t + 1) * H], start=fl, stop=ll)
                nc.tensor.matmul(den_ps[0:1, 0:1], e_tile[:, t:t + 1],
                                 ones_col[:], start=fl, stop=ll)

            rden_sb = cols.tile([1, 1], F32)
            nc.vector.reciprocal(rden_sb[:], den_ps[0:1, 0:1])
            aa_sb = cols.tile([1, H], F32)
            nc.vector.tensor_scalar_mul(aa_sb[:], num_ps[0:1, :], rden_sb[:])

            # attn_weights output = e / den
            rdbc_ps = ps.tile([128, 1], F32, tag="pscol", bufs=3)
            nc.tensor.matmul(rdbc_ps[:], ones_row[:], rden_sb[:], start=True, stop=True)
            rdbc_sb = cols.tile([128, 1], F32)
            nc.vector.tensor_copy(rdbc_sb[:], rdbc_ps[:])
            aw_tile = cols.tile([128, 4], F32)
            nc.vector.tensor_scalar_mul(aw_tile[:], e_tile[:], rdbc_sb[:])
            nc.gpsimd.dma_start(aw_out[:], aw_tile[:])

            # broadcast attn_applied across partitions (PE outer product)
            aabc_ps = ps.tile([128, H], F32, tag="pswide", bufs=2)
            nc.tensor.matmul(aabc_ps[:, 0:512], ones_row[:], aa_sb[0:1, 0:512],
                             start=True, stop=True)
            nc.tensor.matmul(aabc_ps[:, 512:1024], ones_row[:], aa_sb[0:1, 512:1024],
                             start=True, stop=True)

            # ---- comb attn-half, relu ----
            x1 = cols.tile([128, 1], F32)
            prod = scr.tile([128, H], F32, tag="prod", name="prod_c1")
            nc.vector.tensor_mul(prod[:], comb_sb[:, E:EH], aabc_ps[:])
            nc.scalar.activation(dump.broadcast_to(prod[:].shape), prod[:],
                                 AF.Copy, accum_out=x1[:])
            xsum = cols.tile([128, 1], F32)
            nc.vector.tensor_add(xsum[:], x0[:], x1[:])
            x_col = cols.tile([128, 1], F32)
            nc.scalar.activation(x_col[:], xsum[:], AF.Relu, bias=bias_sb[:, 4:5])

            # ---- AllGather x ----
            xT_ps = ps.tile([1, 128], F32, tag="pscol", bufs=3)
            nc.tensor.transpose(xT_ps[:], x_col[:], eye_sb[:])
            xrow_sb = cols.tile([1, 128], F32)
            nc.vector.tensor_copy(xrow_sb[:], xT_ps[:])
            cin1 = dram.tile([1, 128], F32)
            nc.scalar.dma_start(cin1[:], xrow_sb[:])
            cout1 = dram.tile([1, H], F32, addr_space="Shared")
            nc.gpsimd.collective_compute("AllGather", ALU.bypass, replica_groups=rg,
                                         ins=[cin1[:]], outs=[cout1[:]])
            x_bc = wts.tile([128, H], F32)
            nc.scalar.dma_start(x_bc[:], cout1[:].squeeze(0).partition_broadcast(128))

            # ---- gi matvecs ----
            for g in range(3):
                prod = scr.tile([128, H], F32, tag="prod", name=f"prod_gi{g}")
                nc.vector.tensor_mul(prod[:], wih_sb[:, g * H:(g + 1) * H], x_bc[:])
                nc.scalar.activation(dump.broadcast_to(prod[:].shape), prod[:],
                                     AF.Copy, accum_out=gcols[:, g:g + 1])

            # r = sigmoid(gi_r+gh_r+b_r) via 0.5*tanh(0.5*x + b_r/2)+0.5
            rpre = cols.tile([128, 1], F32)
            nc.vector.tensor_add(rpre[:], gcols[:, 0:1], gcols[:, 3:4])
            rth = cols.tile([128, 1], F32)
            nc.scalar.activation(rth[:], rpre[:], AF.Tanh, bias=bias_sb[:, 0:1], scale=0.5)
            r_col = cols.tile([128, 1], F32)
            nc.vector.tensor_scalar(r_col[:], rth[:], 0.5, 0.5, op0=ALU.mult, op1=ALU.add)

            zpre = cols.tile([128, 1], F32)
            nc.vector.tensor_add(zpre[:], gcols[:, 1:2], gcols[:, 4:5])
            zth = cols.tile([128, 1], F32)
            nc.scalar.activation(zth[:], zpre[:], AF.Tanh, bias=bias_sb[:, 1:2], scale=0.5)
            z_col = cols.tile([128, 1], F32)
            nc.vector.tensor_scalar(z_col[:], zth[:], 0.5, 0.5, op0=ALU.mult, op1=ALU.add)

            # n = tanh(gi_n + b_ihn + r*(gh_n + b_hhn))
            t1 = cols.tile([128, 1], F32)
            nc.vector.tensor_scalar_add(t1[:], gcols[:, 5:6], bias_sb[:, 3:4])
            t2 = cols.tile([128, 1], F32)
            nc.vector.tensor_mul(t2[:], r_col[:], t1[:])
            t3 = cols.tile([128, 1], F32)
            nc.vector.tensor_add(t3[:], gcols[:, 2:3], t2[:])
            n_col = cols.tile([128, 1], F32)
            nc.scalar.activation(n_col[:], t3[:], AF.Tanh, bias=bias_sb[:, 2:3])

            # h1 = n + z*(h0 - n)
            d_col = cols.tile([128, 1], F32)
            nc.vector.tensor_sub(d_col[:], bias_sb[:, 5:6], n_col[:])
            zd_col = cols.tile([128, 1], F32)
            nc.vector.tensor_mul(zd_col[:], z_col[:], d_col[:])
            h1_col = cols.tile([128, 1], F32)
            nc.vector.tensor_add(h1_col[:], n_col[:], zd_col[:])

            # ---- AllGather h1 ----
            h1T_ps = ps.tile([1, 128], F32, tag="pscol", bufs=3)
            nc.tensor.transpose(h1T_ps[:], h1_col[:], eye_sb[:])
            h1row_sb = cols.tile([1, 128], F32)
            nc.vector.tensor_copy(h1row_sb[:], h1T_ps[:])
            cin2 = dram.tile([1, 128], F32)
            nc.scalar.dma_start(cin2[:], h1row_sb[:])
            cout2 = dram.tile([1, H], F32, addr_space="Shared")
            nc.gpsimd.collective_compute("AllGather", ALU.bypass, replica_groups=rg,
                                         ins=[cin2[:]], outs=[cout2[:]])
            nc.gpsimd.dma_start(h1_out[:], cout2[:])
            h1_bc = wts.tile([128, H], F32)
            nc.scalar.dma_start(h1_bc[:], cout2[:].squeeze(0).partition_broadcast(128))

            # ---- out projection: z[p,t] = out_W[50p+t] . h1 ----
            wout_v = wout[:].rearrange("(p c) h -> p c h", p=128)
            zraw = cols.tile([128, TK], F32)
            for dd in range(NDMA):
                wt = big.tile([128, DBLK * H], F32, tag="wtile", name=f"wt{dd}")
                nc.sync.dma_start(wt[:].rearrange("p (b h) -> p b h", b=DBLK),
                                  wout_v[:, dd * DBLK:(dd + 1) * DBLK, :])
                for b in range(DBLK):
                    t = dd * DBLK + b
                    prod = scr.tile([128, H], F32, tag="prod", name=f"prod_o{t}")
                    nc.vector.tensor_mul(prod[:], wt[:, b * H:(b + 1) * H], h1_bc[:])
                    nc.scalar.activation(dump.broadcast_to(prod[:].shape), prod[:],
                                         AF.Copy, accum_out=zraw[:, t:t + 1])

            z_tile = cols.tile([128, TK], F32)
            nc.vector.tensor_add(z_tile[:], zraw[:], outb_sb[:])
            nc.scalar.dma_start(z_out[:], z_tile[:])

            # local sum(exp(z))
            sume = cols.tile([128, 1], F32)
            nc.scalar.activation(dump.broadcast_to(z_tile[:].shape), z_tile[:],
                                 AF.Exp, accum_out=sume[:])
            s_ps = ps.tile([1, 1], F32, tag="pscol", bufs=3)
            nc.tensor.matmul(s_ps[:], sume[:], ones_col[:], start=True, stop=True)
            s_sb = cols.tile([1, 1], F32)
            nc.vector.tensor_copy(s_sb[:], s_ps[:])
            nc.scalar.dma_start(s_out[:], s_sb[:])

    nc.compile()
    return nc


def _prep_inputs(input_tok, hidden, encoder_outputs, emb, attn_W, attn_b,
                 comb_W, comb_b, W_ih, W_hh, b_ih, b_hh, out_W, out_b):
    tok = int(np.asarray(input_tok).reshape(-1)[0])
    h0 = np.asarray(hidden, np.float32).reshape(H)
    embedded = np.asarray(emb[tok], np.float32).reshape(E)
    cat1 = np.concatenate([embedded, h0]).reshape(1, EH)

    out_W = np.asarray(out_W, np.float32)
    out_b = np.asarray(out_b, np.float32)
    wout_pad = np.zeros((VP, H), np.float32)
    wout_pad[:V] = out_W
    outb_pad = np.full((VP,), -1e30, np.float32)
    outb_pad[:V] = out_b

    W_ih = np.asarray(W_ih, np.float32)
    W_hh = np.asarray(W_hh, np.float32)
    b_ih = np.asarray(b_ih, np.float32)
    b_hh = np.asarray(b_hh, np.float32)
    comb_W = np.asarray(comb_W, np.float32)
    comb_b = np.asarray(comb_b, np.float32)
    attn_W = np.asarray(attn_W, np.float32)
    attn_b = np.asarray(attn_b, np.float32)
    encoder_outputs = np.ascontiguousarray(np.asarray(encoder_outputs, np.float32))
    eye = np.eye(128, dtype=np.float32)

    in_maps = []
    for k in range(NCORES):
        j = k * 128
        sl = slice(j, j + 128)
        idx = (np.arange(3)[None, :] * H + j + np.arange(128)[:, None]).reshape(-1)
        srows = np.stack([
            0.5 * (b_ih[sl] + b_hh[sl]),
            0.5 * (b_ih[H + j:H + j + 128] + b_hh[H + j:H + j + 128]),
            b_ih[2 * H + j:2 * H + j + 128],
            b_hh[2 * H + j:2 * H + j + 128],
            comb_b[sl],
            h0[sl],
            *attn_b.reshape(128, 4).T,
        ])
        in_maps.append({
            "wout": np.ascontiguousarray(wout_pad[k * VK:(k + 1) * VK]),
            "outb": np.ascontiguousarray(outb_pad[k * VK:(k + 1) * VK].reshape(128, TK)),
            "wih": np.ascontiguousarray(W_ih[idx]),
            "whh": np.ascontiguousarray(W_hh[idx]),
            "combw": np.ascontiguousarray(comb_W[sl]),
            "srows": np.ascontiguousarray(srows.astype(np.float32)),
            "attw": attn_W,
            "enc": encoder_outputs,
            "cat1": cat1,
            "eye": eye,
        })
    return in_maps


def kernel(**inputs):
    global LAST_EXEC_NS
    if "nc" not in _CACHE:
        _CACHE["nc"] = _build()
    nc = _CACHE["nc"]

    in_maps = _prep_inputs(**inputs)
    trace = bool(os.environ.get("BASS_TRACE"))
    if trace:
        _install_axon_profile_hook()
    res = run_bass_kernel_spmd(nc, in_maps, list(range(NCORES)), trace=trace)
    LAST_EXEC_NS = res.exec_time_ns
    outs = res.results

    # z[p, t] = logit for padded-vocab row k*VK + p*TK + t -> plain flatten
    logits = np.concatenate([outs[k]["z"].reshape(-1) for k in range(NCORES)])
    s_total = float(sum(outs[k]["s"][0, 0] for k in range(NCORES)))
    logz = np.float32(np.log(s_total))
    logp = (logits[:V] - logz).reshape(1, V).astype(np.float32)
    h1 = outs[0]["h1f"].reshape(1, 1, H).astype(np.float32)
    # aw[p, t] = attention weight for position 4p+t -> plain flatten
    aw = outs[0]["aw"].reshape(1, L).astype(np.float32)
    return (logp, h1, aw)


# revision 7
# speedup vs baseline: 1.2622x; 1.2622x over previous
"""AttnDecoderRNN single-step decoder on 8 Trainium2 NeuronCores.

Sharding strategy:
  - attention and the comb layer are replicated on every core (small, on
    the critical path; replication avoids collectives).
  - GRU weights are sharded by output rows: core k owns H-slice
    j = [128k, 128k+128). One tiny AllGather ([1,128] -> [1,1024])
    stitches the new hidden state h1 back together.
  - out_W is sharded by vocab rows (6400 padded rows per core), host-cast
    to bf16 (weight-only; ~2e-4 effect on logp). Each core computes its
    logit shard plus a local sum(exp(z)); the host epilogue combines the 8
    partial sums into log Z and subtracts (log_softmax).

Matvecs run as DVE tensor_mul (weights-tile x broadcast-vector) reduced
along the free dim either on ScalarE (activation Copy accum_out) or on DVE
(tensor_reduce) - the big phase splits reductions across both engines so
neither stalls the multiply stream.

Row->partition mapping keeps DMA descriptors large and contiguous:
vocab rows in slab order (partition p owns rows p*50+t), attention rows
p*4+t, GRU/comb rows interleaved c*128+p.
"""

import os
import numpy as np
import ml_dtypes

import concourse.bass as bass
import concourse.bacc as bacc
import concourse.mybir as mybir
import concourse.tile as tile
from concourse.bass_utils import run_bass_kernel_spmd

F32 = mybir.dt.float32
BF16 = mybir.dt.bfloat16
AF = mybir.ActivationFunctionType
ALU = mybir.AluOpType
AX = mybir.AxisListType

V, E, H, L = 50257, 512, 1024, 512
EH = E + H
NCORES = 8
VP = 51200            # vocab padded to 8 * 6400
VK = VP // NCORES     # 6400 vocab rows per core
TK = VK // 128        # 50 vocab rows per partition per core
DBLK = 5              # rows-per-partition fetched per big DMA
NDMA = TK // DBLK     # 10 big DMAs per core
DVE_RED = set(range(3, TK, 4))   # big tiles whose reduction runs on DVE

_CACHE = {}
LAST_EXEC_NS = None


def _install_axon_profile_hook():
    """Provide antenv.axon_hooks (NTFF profiling via the axon .so) when the
    image lacks it, so run_bass_kernel_spmd(trace=True) can measure HW time.
    No-op if unavailable; the untraced path never needs it."""
    import sys
    import types
    import ctypes
    import contextlib

    if "antenv.axon_hooks" in sys.modules:
        return
    so_path = "/opt/axon/libaxon_pjrt.so"
    if not os.path.exists(so_path):
        return
    try:
        lib = ctypes.CDLL(so_path)
        if not hasattr(lib, "axon_start_nrt_profile"):
            return
        lib.axon_start_nrt_profile.argtypes = [ctypes.POINTER(ctypes.c_int64),
                                               ctypes.c_size_t]
        lib.axon_start_nrt_profile.restype = ctypes.c_int64
        lib.axon_stop_nrt_profile.argtypes = [ctypes.c_char_p]
        lib.axon_stop_nrt_profile.restype = ctypes.c_int64

        @contextlib.contextmanager
        def _hook(output_dir, device_ids):
            import jax
            jax.devices()
            if device_ids:
                ids = (ctypes.c_int64 * len(device_ids))(*device_ids)
                rc = lib.axon_start_nrt_profile(ids, len(device_ids))
            else:
                rc = lib.axon_start_nrt_profile(None, 0)
            if rc != 0:
                raise RuntimeError(f"axon_start_nrt_profile rc={rc}")
            try:
                yield
            finally:
                n = lib.axon_stop_nrt_profile(str(output_dir).encode())
                print(f"profile: {n} file(s) written to {output_dir}", file=sys.stderr)

        mod = types.ModuleType("antenv.axon_hooks")
        mod.get_axon_ntff_profile_hook = lambda: _hook
        mod.set_axon_ntff_profile_hook = lambda h: None
        sys.modules["antenv.axon_hooks"] = mod

        import concourse.bass_utils as bu
        bu.upload_artifacts = lambda tmpdir: tmpdir
    except Exception:
        pass


def _build():
    nc = bacc.Bacc("TRN2", target_bir_lowering=False, debug=False,
                   num_devices=NCORES)

    wout = nc.dram_tensor("wout", [VK, H], BF16, kind="ExternalInput")
    outb = nc.dram_tensor("outb", [128, TK], F32, kind="ExternalInput")
    wih = nc.dram_tensor("wih", [384, H], F32, kind="ExternalInput")
    whh = nc.dram_tensor("whh", [384, H], F32, kind="ExternalInput")
    combw = nc.dram_tensor("combw", [1024, EH], F32, kind="ExternalInput")
    combb = nc.dram_tensor("combb", [128, 8], F32, kind="ExternalInput")
    srows = nc.dram_tensor("srows", [9, 128], F32, kind="ExternalInput")
    attw = nc.dram_tensor("attw", [L, EH], F32, kind="ExternalInput")
    enc = nc.dram_tensor("enc", [L, H], F32, kind="ExternalInput")
    cat1 = nc.dram_tensor("cat1", [1, EH], F32, kind="ExternalInput")
    eye = nc.dram_tensor("eye", [128, 128], F32, kind="ExternalInput")

    z_out = nc.dram_tensor("z", [128, TK], F32, kind="ExternalOutput")
    s_out = nc.dram_tensor("s", [1, 1], F32, kind="ExternalOutput")
    h1_out = nc.dram_tensor("h1f", [1, H], F32, kind="ExternalOutput")
    aw_out = nc.dram_tensor("aw", [128, 4], F32, kind="ExternalOutput")

    rg = [list(range(NCORES))]

    with tile.TileContext(nc) as tc:
        with (
            tc.tile_pool(name="wts", bufs=1) as wts,
            tc.tile_pool(name="big", bufs=3) as big,
            tc.tile_pool(name="scr", bufs=3) as scr,
            tc.tile_pool(name="cols", bufs=1) as cols,
            tc.tile_pool(name="ps", bufs=1, space="PSUM") as ps,
            tc.tile_pool(name="dram", bufs=1, space="DRAM") as dram,
        ):
            # ---- weight loads (sync HWDGE ring, in consumption order) ----
            attn_sb = wts.tile([128, 4 * EH], F32)
            attw_v = attw[:].rearrange("(p t) f -> p t f", p=128)
            for t in range(4):
                nc.sync.dma_start(attn_sb[:, t * EH:(t + 1) * EH], attw_v[:, t, :])
            # comb is replicated; rows interleaved c*128+p; emb half first
            comb_sb = wts.tile([128, 8 * EH], F32)
            comb_v2 = comb_sb[:].rearrange("p (c f) -> p c f", c=8)
            combw_v = combw[:].rearrange("(c p) f -> p c f", p=128)
            nc.sync.dma_start(comb_v2[:, :, 0:E], combw_v[:, :, 0:E])
            whh_sb = wts.tile([128, 3 * H], F32)
            nc.sync.dma_start(whh_sb[:].rearrange("p (g h) -> p g h", g=3),
                              whh[:].rearrange("(p g) h -> p g h", p=128))
            enc_sb = wts.tile([128, 4 * H], F32)
            nc.sync.dma_start(enc_sb[:].rearrange("p (t h) -> p t h", t=4),
                              enc[:].rearrange("(p t) h -> p t h", p=128))
            nc.sync.dma_start(comb_v2[:, :, E:EH], combw_v[:, :, E:EH])
            wih_sb = wts.tile([128, 3 * H], F32)
            nc.sync.dma_start(wih_sb[:].rearrange("p (g h) -> p g h", g=3),
                              wih[:].rearrange("(p g) h -> p g h", p=128))

            # ---- tiny loads / consts (scalar HWDGE = low latency) ----
            cat1_bc = wts.tile([128, EH], F32)
            nc.scalar.dma_start(cat1_bc[:], cat1[:].squeeze(0).partition_broadcast(128))
            eye_sb = cols.tile([128, 128], F32)
            nc.scalar.dma_start(eye_sb[:], eye[:])
            srows_sb = cols.tile([9, 128], F32)
            nc.scalar.dma_start(srows_sb[:], srows[:])
            outb_sb = cols.tile([128, TK], F32)
            nc.scalar.dma_start(outb_sb[:], outb[:])
            combb_sb = cols.tile([128, 8], F32)
            nc.scalar.dma_start(combb_sb[:], combb[:])
            ones_col = cols.tile([128, 1], F32)
            nc.gpsimd.memset(ones_col[:], 1.0)
            ones_row = cols.tile([1, 128], F32)
            nc.gpsimd.memset(ones_row[:], 1.0)

            # bias rows -> per-partition columns
            # cols: 0 r-bias/2, 1 z-bias/2, 2 b_ihn, 3 b_hhn, 4 h0, 5..8 attn_b
            colsT_ps = ps.tile([128, 9], F32, tag="pscol", bufs=3)
            nc.tensor.transpose(colsT_ps[:], srows_sb[:], eye_sb[0:9, 0:9])
            bias_sb = cols.tile([128, 9], F32)
            nc.vector.tensor_copy(bias_sb[:], colsT_ps[:])

            dump = cols.tile([128, 1], F32)

            # ---- attention matvec: za[p,t] = attn_W[4p+t] . cat1 ----
            zaraw = cols.tile([128, 4], F32)
            for t in range(4):
                prod = scr.tile([128, EH], F32, tag="prod", name=f"prod_a{t}")
                nc.vector.tensor_mul(prod[:], attn_sb[:, t * EH:(t + 1) * EH], cat1_bc[:])
                nc.scalar.activation(dump.broadcast_to(prod[:].shape), prod[:],
                                     AF.Copy, accum_out=zaraw[:, t:t + 1])

            # ---- comb embedding-half early (independent of attention) ----
            x0 = cols.tile([128, 8], F32)
            for c in range(8):
                prod = scr.tile([128, E], F32, tag="prod", name=f"prod_ce{c}")
                nc.vector.tensor_mul(prod[:], comb_v2[:, c, 0:E], cat1_bc[:, 0:E])
                nc.scalar.activation(dump.broadcast_to(prod[:].shape), prod[:],
                                     AF.Copy, accum_out=x0[:, c:c + 1])

            # ---- gh matvecs early (independent of attention) ----
            gcols = cols.tile([128, 6], F32)   # gi r/z/n, gh r/z/n
            for g in range(3):
                prod = scr.tile([128, H], F32, tag="prod", name=f"prod_gh{g}")
                nc.vector.tensor_mul(prod[:], whh_sb[:, g * H:(g + 1) * H],
                                     cat1_bc[:, E:EH])
                nc.scalar.activation(dump.broadcast_to(prod[:].shape), prod[:],
                                     AF.Copy, accum_out=gcols[:, 3 + g:4 + g])

            # ---- attention softmax-weighted sum ----
            e_tile = cols.tile([128, 4], F32)
            for t in range(4):
                nc.scalar.activation(e_tile[:, t:t + 1], zaraw[:, t:t + 1], AF.Exp,
                                     bias=bias_sb[:, 5 + t:6 + t])
            num_ps = ps.tile([1, H], F32, tag="pswide", bufs=2)
            den_ps = ps.tile([1, 1], F32, tag="pscol", bufs=3)
            for t in range(4):
                fl = (t == 0)
                ll = (t == 3)
                nc.tensor.matmul(num_ps[0:1, 0:512], e_tile[:, t:t + 1],
                                 enc_sb[:, t * H:t * H + 512], start=fl, stop=ll)
                nc.tensor.matmul(num_ps[0:1, 512:1024], e_tile[:, t:t + 1],
                                 enc_sb[:, t * H + 512:(t + 1) * H], start=fl, stop=ll)
                nc.tensor.matmul(den_ps[0:1, 0:1], e_tile[:, t:t + 1],
                                 ones_col[:], start=fl, stop=ll)

            rden_sb = cols.tile([1, 1], F32)
            nc.vector.reciprocal(rden_sb[:], den_ps[0:1, 0:1])
            aa_sb = cols.tile([1, H], F32)
            nc.vector.tensor_scalar_mul(aa_sb[:], num_ps[0:1, :], rden_sb[:])

            # attn_weights output = e / den
            rdbc_ps = ps.tile([128, 1], F32, tag="pscol", bufs=3)
            nc.tensor.matmul(rdbc_ps[:], ones_row[:], rden_sb[:], start=True, stop=True)
            rdbc_sb = cols.tile([128, 1], F32)
            nc.vector.tensor_copy(rdbc_sb[:], rdbc_ps[:])
            aw_tile = cols.tile([128, 4], F32)
            nc.vector.tensor_scalar_mul(aw_tile[:], e_tile[:], rdbc_sb[:])
            nc.gpsimd.dma_start(aw_out[:], aw_tile[:])

            # broadcast attn_applied across partitions (PE outer product)
            aabc_ps = ps.tile([128, H], F32, tag="pswide", bufs=2)
            nc.tensor.matmul(aabc_ps[:, 0:512], ones_row[:], aa_sb[0:1, 0:512],
                             start=True, stop=True)
            nc.tensor.matmul(aabc_ps[:, 512:1024], ones_row[:], aa_sb[0:1, 512:1024],
                             start=True, stop=True)

            # ---- comb attn-half; x = relu(x0 + x1 + comb_b), full H locally ----
            x1 = cols.tile([128, 8], F32)
            for c in range(8):
                prod = scr.tile([128, H], F32, tag="prod", name=f"prod_ca{c}")
                nc.vector.tensor_mul(prod[:], comb_v2[:, c, E:EH], aabc_ps[:])
                nc.scalar.activation(dump.broadcast_to(prod[:].shape), prod[:],
                                     AF.Copy, accum_out=x1[:, c:c + 1])
            xsum = cols.tile([128, 8], F32)
            nc.vector.tensor_add(xsum[:], x0[:], x1[:])
            xsb = cols.tile([128, 8], F32)
            nc.vector.tensor_add(xsb[:], xsum[:], combb_sb[:])
            x_act = cols.tile([128, 8], F32)
            nc.scalar.activation(x_act[:], xsb[:], AF.Relu)

            # x: [128,8] cols (x[c*128+p]) -> row [1,1024] via PE transpose
            xT_ps = ps.tile([8, 128], F32, tag="pscol", bufs=3)
            nc.tensor.transpose(xT_ps[:], x_act[:], eye_sb[:])
            xT_sb = cols.tile([8, 128], F32)
            nc.vector.tensor_copy(xT_sb[:], xT_ps[:])
            xfull = dram.tile([8, 128], F32)
            nc.scalar.dma_start(xfull[:], xT_sb[:])
            x_bc = wts.tile([128, H], F32)
            nc.scalar.dma_start(x_bc[:], xfull[:].rearrange("a b -> (a b)")
                                .partition_broadcast(128))

            # ---- gi matvecs ----
            for g in range(3):
                prod = scr.tile([128, H], F32, tag="prod", name=f"prod_gi{g}")
                nc.vector.tensor_mul(prod[:], wih_sb[:, g * H:(g + 1) * H], x_bc[:])
                nc.scalar.activation(dump.broadcast_to(prod[:].shape), prod[:],
                                     AF.Copy, accum_out=gcols[:, g:g + 1])

            # r = sigmoid(gi_r+gh_r+b_r) via 0.5*tanh(0.5*x + b_r/2)+0.5
            rpre = cols.tile([128, 1], F32)
            nc.vector.tensor_add(rpre[:], gcols[:, 0:1], gcols[:, 3:4])
            rth = cols.tile([128, 1], F32)
            nc.scalar.activation(rth[:], rpre[:], AF.Tanh, bias=bias_sb[:, 0:1], scale=0.5)
            r_col = cols.tile([128, 1], F32)
            nc.vector.tensor_scalar(r_col[:], rth[:], 0.5, 0.5, op0=ALU.mult, op1=ALU.add)

            zpre = cols.tile([128, 1], F32)
            nc.vector.tensor_add(zpre[:], gcols[:, 1:2], gcols[:, 4:5])
            zth = cols.tile([128, 1], F32)
            nc.scalar.activation(zth[:], zpre[:], AF.Tanh, bias=bias_sb[:, 1:2], scale=0.5)
            z_col = cols.tile([128, 1], F32)
            nc.vector.tensor_scalar(z_col[:], zth[:], 0.5, 0.5, op0=ALU.mult, op1=ALU.add)

            # n = tanh(gi_n + b_ihn + r*(gh_n + b_hhn))
            t1 = cols.tile([128, 1], F32)
            nc.vector.tensor_scalar_add(t1[:], gcols[:, 5:6], bias_sb[:, 3:4])
            t2 = cols.tile([128, 1], F32)
            nc.vector.tensor_mul(t2[:], r_col[:], t1[:])
            t3 = cols.tile([128, 1], F32)
            nc.vector.tensor_add(t3[:], gcols[:, 2:3], t2[:])
            n_col = cols.tile([128, 1], F32)
            nc.scalar.activation(n_col[:], t3[:], AF.Tanh, bias=bias_sb[:, 2:3])

            # h1 = n + z*(h0 - n)
            d_col = cols.tile([128, 1], F32)
            nc.vector.tensor_sub(d_col[:], bias_sb[:, 4:5], n_col[:])
            zd_col = cols.tile([128, 1], F32)
            nc.vector.tensor_mul(zd_col[:], z_col[:], d_col[:])
            h1_col = cols.tile([128, 1], F32)
            nc.vector.tensor_add(h1_col[:], n_col[:], zd_col[:])

            # ---- AllGather h1 (the only collective) ----
            h1T_ps = ps.tile([1, 128], F32, tag="pscol", bufs=3)
            nc.tensor.transpose(h1T_ps[:], h1_col[:], eye_sb[:])
            h1row_sb = cols.tile([1, 128], F32)
            nc.vector.tensor_copy(h1row_sb[:], h1T_ps[:])
            cin2 = dram.tile([1, 128], F32)
            nc.scalar.dma_start(cin2[:], h1row_sb[:])
            cout2 = dram.tile([1, H], F32, addr_space="Shared")
            nc.gpsimd.collective_compute("AllGather", ALU.bypass, replica_groups=rg,
                                         ins=[cin2[:]], outs=[cout2[:]])
            nc.gpsimd.dma_start(h1_out[:], cout2[:])
            # broadcast + cast to bf16 in one SWDGE DMA
            h1_bc = wts.tile([128, H], BF16)
            nc.gpsimd.dma_start(h1_bc[:], cout2[:].squeeze(0).partition_broadcast(128))

            # ---- out projection: z[p,t] = out_W[50p+t] . h1 (bf16) ----
            wout_v = wout[:].rearrange("(p c) h -> p c h", p=128)
            zraw = cols.tile([128, TK], F32)
            for dd in range(NDMA):
                wt = big.tile([128, DBLK * H], BF16, tag="wtile", name=f"wt{dd}")
                nc.sync.dma_start(wt[:].rearrange("p (b h) -> p b h", b=DBLK),
                                  wout_v[:, dd * DBLK:(dd + 1) * DBLK, :])
                for b in range(DBLK):
                    t = dd * DBLK + b
                    prod = scr.tile([128, H], BF16, tag="prodb", name=f"prod_o{t}")
                    nc.vector.tensor_mul(prod[:], wt[:, b * H:(b + 1) * H], h1_bc[:])
                    if t in DVE_RED:
                        nc.vector.reduce_sum(zraw[:, t:t + 1], prod[:], axis=AX.X)
                    else:
                        nc.scalar.activation(dump.broadcast_to(prod[:].shape), prod[:],
                                             AF.Copy, accum_out=zraw[:, t:t + 1])

            z_tile = cols.tile([128, TK], F32)
            nc.vector.tensor_add(z_tile[:], zraw[:], outb_sb[:])
            nc.scalar.dma_start(z_out[:], z_tile[:])

            # local sum(exp(z))
            sume = cols.tile([128, 1], F32)
            nc.scalar.activation(dump.broadcast_to(z_tile[:].shape), z_tile[:],
                                 AF.Exp, accum_out=sume[:])
            s_ps = ps.tile([1, 1], F32, tag="pscol", bufs=3)
            nc.tensor.matmul(s_ps[:], sume[:], ones_col[:], start=True, stop=True)
            s_sb = cols.tile([1, 1], F32)
            nc.vector.tensor_copy(s_sb[:], s_ps[:])
            nc.scalar.dma_start(s_out[:], s_sb[:])

    nc.compile()
    return nc


def _prep_inputs(input_tok, hidden, encoder_outputs, emb, attn_W, attn_b,
                 comb_W, comb_b, W_ih, W_hh, b_ih, b_hh, out_W, out_b):
    tok = int(np.asarray(input_tok).reshape(-1)[0])
    h0 = np.asarray(hidden, np.float32).reshape(H)
    embedded = np.asarray(emb[tok], np.float32).reshape(E)
    cat1 = np.concatenate([embedded, h0]).reshape(1, EH)

    out_W = np.asarray(out_W, np.float32)
    out_b = np.asarray(out_b, np.float32)
    wout_pad = np.zeros((VP, H), ml_dtypes.bfloat16)
    wout_pad[:V] = out_W.astype(ml_dtypes.bfloat16)
    outb_pad = np.full((VP,), -1e30, np.float32)
    outb_pad[:V] = out_b

    W_ih = np.asarray(W_ih, np.float32)
    W_hh = np.asarray(W_hh, np.float32)
    b_ih = np.asarray(b_ih, np.float32)
    b_hh = np.asarray(b_hh, np.float32)
    comb_W = np.asarray(comb_W, np.float32)
    comb_b = np.asarray(comb_b, np.float32)
    attn_W = np.asarray(attn_W, np.float32)
    attn_b = np.asarray(attn_b, np.float32)
    encoder_outputs = np.ascontiguousarray(np.asarray(encoder_outputs, np.float32))
    eye = np.eye(128, dtype=np.float32)

    # comb rows interleaved: DRAM row c*128+p holds comb_W row c*128+p
    # (natural order), device view does the slabbing.
    combw_full = np.ascontiguousarray(comb_W)
    combb_dev = np.ascontiguousarray(comb_b.reshape(8, 128).T.astype(np.float32))

    in_maps = []
    for k in range(NCORES):
        j = k * 128
        sl = slice(j, j + 128)
        idx = (np.arange(3)[None, :] * H + j + np.arange(128)[:, None]).reshape(-1)
        srows = np.stack([
            0.5 * (b_ih[sl] + b_hh[sl]),
            0.5 * (b_ih[H + j:H + j + 128] + b_hh[H + j:H + j + 128]),
            b_ih[2 * H + j:2 * H + j + 128],
            b_hh[2 * H + j:2 * H + j + 128],
            h0[sl],
            *attn_b.reshape(128, 4).T,
        ])
        in_maps.append({
            "wout": np.ascontiguousarray(wout_pad[k * VK:(k + 1) * VK]),
            "outb": np.ascontiguousarray(outb_pad[k * VK:(k + 1) * VK].reshape(128, TK)),
            "wih": np.ascontiguousarray(W_ih[idx]),
            "whh": np.ascontiguousarray(W_hh[idx]),
            "combw": combw_full,
            "combb": combb_dev,
            "srows": np.ascontiguousarray(srows.astype(np.float32)),
            "attw": attn_W,
            "enc": encoder_outputs,
            "cat1": cat1,
            "eye": eye,
        })
    return in_maps


def kernel(**inputs):
    global LAST_EXEC_NS
    if "nc" not in _CACHE:
        _CACHE["nc"] = _build()
    nc = _CACHE["nc"]

    in_maps = _prep_inputs(**inputs)
    trace = bool(os.environ.get("BASS_TRACE"))
    if trace:
        _install_axon_profile_hook()
    res = run_bass_kernel_spmd(nc, in_maps, list(range(NCORES)), trace=trace)
    LAST_EXEC_NS = res.exec_time_ns
    outs = res.results

    # z[p, t] = logit for padded-vocab row k*VK + p*TK + t -> plain flatten
    logits = np.concatenate([outs[k]["z"].reshape(-1) for k in range(NCORES)])
    s_total = float(sum(outs[k]["s"][0, 0] for k in range(NCORES)))
    logz = np.float32(np.log(s_total))
    logp = (logits[:V] - logz).reshape(1, V).astype(np.float32)
    h1 = outs[0]["h1f"].reshape(1, 1, H).astype(np.float32)
    # aw[p, t] = attention weight for position 4p+t -> plain flatten
    aw = outs[0]["aw"].reshape(1, L).astype(np.float32)
    return (logp, h1, aw)
